# revision 43
# baseline (speedup 1.0000x reference)
"""Trainium2 Bass kernel for AttentionNet:
out[b,h,i,j] = relu(sum_d w2[d] * Xf[b,h,i,d] * Yf[b,h,j,d] + b2)
where Xf = X @ W1.T + b1, Yf = Y @ W1.T + b1.

Shapes (hardcoded): X,Y [8, 4, 1024, 64] f32; W1 [64,64]; b1,w2 [64]; b2 [].
Sharding: data-parallel over the fused B*H=32 head dim -> 4 heads per core
across 8 NeuronCores; W1/b1/w2/b2 replicated.

Device plan per core (4 heads = 2 head-pairs), compute in bf16 on the
PE with fp32 PSUM accumulation (norm rel err ~4e-3):
- heads are processed in pairs packed into the two 64-row halves of the
  128-partition dim, so every K=64 matmul runs 2x concurrent on the PE
  via tile_position row groups.
- inputs load in natural layout (one DMA per pair-tensor, 2 KiB
  contiguous per partition); a DVE cast to bf16 permutes the free dims
  so each r0-block is a contiguous [128, 128] PE-transpose input. The
  transposes emit columns in i = 8p + r0 order: the X side keeps that
  order (the output DMA's partition stride undoes it); the Y side is
  unscrambled to j-contiguous in the PSUM->SBUF evacuation copy.
- lin1 (W1.T stacked twice) + bias/scale fuse into the stage-1
  PSUM->SBUF copies: A.T = (Xf.T + b1)*w2, B.T = Yf.T + b1.
- scores: lhsT = A.T[64, 128-block], rhs = B.T[64, 512-chunk];
  relu(x + b2) on the PSUM evacuation, alternating ACT/DVE; output DMA
  on the sync HWDGE ring while input loads use the scalar ring.
- all input loads are issued up front (hidden under the prologue), the
  PE is pre-warmed past the HAM clock gate with dummy matmuls, and
  pair N+1's stage-1 chunks are threaded between pair N's score blocks
  so the 433 GB/s output stream never stalls.
"""

import ml_dtypes
import numpy as np
from contextlib import ExitStack

import concourse.bass as bass
import concourse.tile as tile
from concourse import bacc, mybir
from concourse.bass_utils import run_bass_kernel_spmd

# If the caller's environment sets BASS_TRACE, run_bass_kernel_spmd's
# axon trace path imports antenv.axon_hooks, which not every image
# ships. Register a fallback so a stray BASS_TRACE can't crash the run
# (a None hook makes bass_utils skip tracing gracefully).
try:
    import antenv.axon_hooks  # noqa: F401
except ImportError:
    import sys
    import types

    _hooks = types.ModuleType("antenv.axon_hooks")
    _hooks._hook = None

    def _get_hook():
        return _hooks._hook

    def _set_hook(h):
        _hooks._hook = h

    _hooks.get_axon_ntff_profile_hook = _get_hook
    _hooks.set_axon_ntff_profile_hook = _set_hook
    sys.modules["antenv.axon_hooks"] = _hooks

B, H, L, D = 8, 4, 1024, 64
NCORES = 8
HPC = (B * H) // NCORES  # heads per core = 4

F32 = mybir.dt.float32
MM_DT = mybir.dt.bfloat16


def _mm(ap):
    """Matmul-operand view; with bf16 tiles the cast happens in the
    producing op, so this is the identity."""
    return ap


LAST_RESULT = None
_CACHED_NC = None


def _build():
    nc = bacc.Bacc()
    Xd = nc.declare_dram_parameter("X", [HPC, L, D], F32, isOutput=False)
    Yd = nc.declare_dram_parameter("Y", [HPC, L, D], F32, isOutput=False)
    W1T2d = nc.declare_dram_parameter("W1T2", [128, D], MM_DT, isOutput=False)
    Cd = nc.declare_dram_parameter("CONSTS", [128, 4], F32, isOutput=False)
    Idd = nc.declare_dram_parameter("IDENT", [128, 128], MM_DT, isOutput=False)
    Od = nc.declare_dram_parameter("OUT", [HPC, L, L], F32, isOutput=True)

    AF = mybir.ActivationFunctionType

    with tile.TileContext(nc) as tc, ExitStack() as ctx:
        cpool = ctx.enter_context(tc.tile_pool(name="consts", bufs=1))
        xin_pool = ctx.enter_context(tc.tile_pool(name="xin", bufs=4))
        xbf_pool = ctx.enter_context(tc.tile_pool(name="xbf", bufs=4))
        xt_pool = ctx.enter_context(tc.tile_pool(name="xt", bufs=4))
        ab_pool = ctx.enter_context(tc.tile_pool(name="ab", bufs=4))
        out_pool = ctx.enter_context(tc.tile_pool(name="out", bufs=8))
        pt_pool = ctx.enter_context(tc.tile_pool(name="pt", bufs=2, space="PSUM"))
        pf_pool = ctx.enter_context(tc.tile_pool(name="pf", bufs=2, space="PSUM"))
        ps_pool = ctx.enter_context(tc.tile_pool(name="ps", bufs=2, space="PSUM"))

        def load_pair_tensor(pair, nm, src):
            # natural-layout load -- one DMA, 2 KiB contiguous per
            # partition: xin[p, (s r d)] = src[h0+s, 8p + r, d]
            h0 = 2 * pair
            xin = xin_pool.tile([128, 8 * 2 * D], F32, tag=f"xin{pair}{nm}")
            nc.scalar.dma_start(
                xin[:, :].rearrange("p (s r d) -> p s r d", s=2, r=8),
                src[h0 : h0 + 2, :, :].rearrange("s (p r) d -> p s r d", r=8),
            )
            return xin

        # pair-0 input loads first (they gate the whole prologue), then
        # the constants, then pair-1 prefetch.
        ident = cpool.tile([128, 128], MM_DT, tag="ident")
        nc.scalar.dma_start(ident[:, :], Idd[:, :])
        loads = {}
        loads[(0, "b")] = load_pair_tensor(0, "b", Yd)
        loads[(0, "a")] = load_pair_tensor(0, "a", Xd)
        w1t2 = cpool.tile([128, D], MM_DT, tag="w1t2")
        nc.scalar.dma_start(w1t2[:, :], W1T2d[:, :])
        consts = cpool.tile([128, 4], F32, tag="consts")
        nc.scalar.dma_start(consts[:, :], Cd[:, :])
        for pair in range(1, HPC // 2):
            loads[(pair, "b")] = load_pair_tensor(pair, "b", Yd)
            loads[(pair, "a")] = load_pair_tensor(pair, "a", Xd)
        # consts columns: 0 = b1*w2 (stacked 2x), 1 = w2 (2x), 2 = b1 (2x),
        # 3 = b2 broadcast
        biasx = consts[:, 0:1]
        scalex = consts[:, 1:2]
        biasy = consts[:, 2:3]
        b2col = consts[:, 3:4]

        # Warm the PE while input loads are in flight: the HAM clock
        # gate needs ~3.4 us of sustained matmul activity to lift the PE
        # from 1.2 to 2.4 GHz, and the prologue would otherwise run the
        # whole stage-1 chain cold. Transpose-mode does not count as
        # PE-busy for HAM, so use real matmuls on the identity tile.
        warm = ps_pool.tile([128, 128], F32, tag="ps")
        for _ in range(40):
            nc.tensor.matmul(
                warm[:, :], lhsT=ident[:, :], rhs=ident[:, :],
                start=True, stop=True,
            )

        def stage1_chunks(pair, ab):
            """Yield stage-1 work as small closures so pair N+1's chain
            can be threaded between pair N's score blocks (the PE runs
            its queue in order; a monolithic stage-1 after the last
            score block would stall the output stream)."""
            for nm, bias_ap, scale_ap in (
                ("b", biasy, None),
                ("a", biasx, scalex),
            ):
                xin = loads[(pair, nm)]
                xbf = xbf_pool.tile([128, 8 * 2 * D], MM_DT, tag="xbf")
                pt = pt_pool.tile([128, L], MM_DT, tag="pt")
                xt = xt_pool.tile([128, L], MM_DT, tag="xt")
                dst = ab_pool.tile([128, L], MM_DT, tag="ab")
                ab[nm] = dst

                def chunk_a(nm=nm, xin=xin, xbf=xbf, pt=pt, xt=xt):
                    # cast permutes free dims to (r, s, d) so each
                    # r0-block is a contiguous [128, (s d)] transpose
                    # input
                    nc.vector.tensor_copy(
                        xbf[:, :].rearrange("p (r s d) -> p r s d", s=2, r=8),
                        xin[:, :].rearrange("p (s r d) -> p r s d", s=2, r=8),
                    )
                    # PE transpose block r0 -> [128, 128] PSUM: rows
                    # 0-63 = head0 d's, 64-127 = head1 d's; columns are
                    # i = 8p + r0
                    for k in range(8):
                        nc.tensor.transpose(
                            pt[:, bass.ts(k, 128)],
                            xbf[:, bass.ts(k, 128)],
                            ident[:, :],
                        )
                    if nm == "a":
                        # X keeps the scrambled i = 8p + r0 column
                        # order; the out-DMA partition stride undoes it.
                        nc.vector.tensor_copy(_mm(xt[:, :]), pt[:, :])
                    else:
                        # Y must be j-contiguous (scores rhs / output
                        # free dim follow its column order): unscramble
                        # the free-dim permutation in the evacuation
                        # copy -- xt[k, 8p + r] = pt[k, r*128 + p].
                        nc.vector.tensor_copy(
                            xt[:, :].rearrange("k (p r) -> k p r", r=8),
                            pt[:, :].rearrange("k (r p) -> k p r", p=128),
                        )

                def chunk_b(
                    xt=xt, dst=dst, bias_ap=bias_ap, scale_ap=scale_ap
                ):
                    # lin1 for both heads concurrently on row groups
                    # 0-1 / 2-3, one [128, 512] PSUM tile per n-chunk;
                    # bias/scale fused on the PSUM->SBUF copy:
                    # (x + b1) * w2 resp. (y + b1)
                    for n in range(2):
                        pf = pf_pool.tile([128, 512], F32, tag="pf")
                        for s in range(2):
                            rows = slice(64 * s, 64 * s + 64)
                            nc.tensor.matmul(
                                pf[rows, :],
                                lhsT=_mm(w1t2[rows, :]),
                                rhs=_mm(xt[rows, bass.ts(n, 512)]),
                                start=True,
                                stop=True,
                                tile_position=(64 * s, 64 * s),
                            )
                        nc.scalar.activation(
                            _mm(dst[:, bass.ts(n, 512)]),
                            pf[:, :],
                            AF.Identity,
                            bias=bias_ap,
                            scale=scale_ap if scale_ap is not None else 1.0,
                        )

                yield chunk_a
                yield chunk_b

        relu_ctr = 0
        npairs = HPC // 2
        ab_cur = {}
        for ch in stage1_chunks(0, ab_cur):
            ch()
        for pair in range(npairs):
            h0 = 2 * pair
            ab = ab_cur
            ab_next = {}
            next_chunks = (
                list(stage1_chunks(pair + 1, ab_next))
                if pair + 1 < npairs
                else []
            )
            # scores: out[i, j] = sum_d A.T[d, i] * B.T[d, j]; the two
            # heads of the pair run on disjoint PE row groups. lhsT
            # block m covers rows i = 8p + m; rhs is j-contiguous.
            for m in range(8):
                if next_chunks and 4 <= m < 4 + len(next_chunks):
                    next_chunks[m - 4]()
                for s in range(2):
                    rows = slice(64 * s, 64 * s + 64)
                    ps = ps_pool.tile([128, L], F32, tag="ps")
                    for n in range(2):
                        nc.tensor.matmul(
                            ps[:, bass.ts(n, 512)],
                            lhsT=_mm(ab["a"][rows, bass.ts(m, 128)]),
                            rhs=_mm(ab["b"][rows, bass.ts(n, 512)]),
                            start=True,
                            stop=True,
                            tile_position=(64 * s, 0),
                        )
                    o = out_pool.tile([128, L], F32, tag="o")
                    # A.T block m has columns i = 8p + m, so scores rows
                    # scatter back with partition stride 8.
                    od = Od[h0 + s, :, :].rearrange("(p r) j -> p r j", r=8)[
                        :, m, :
                    ]
                    if relu_ctr in (0, 31):
                        # first block gates the stream start, last block
                        # gates the tail drain: split them in half across
                        # both engines so their DMAs issue sooner.
                        for half in range(2):
                            js = bass.ts(half, 512)
                            if half == 0:
                                nc.scalar.activation(
                                    o[:, js],
                                    ps[:, js],
                                    AF.Relu,
                                    bias=b2col,
                                    scale=1.0,
                                )
                            else:
                                nc.vector.tensor_scalar(
                                    o[:, js],
                                    ps[:, js],
                                    b2col,
                                    0.0,
                                    mybir.AluOpType.add,
                                    mybir.AluOpType.max,
                                )
                            nc.sync.dma_start(od[:, js], o[:, js])
                    else:
                        if relu_ctr % 2 == 0:
                            nc.scalar.activation(
                                o[:, :], ps[:, :], AF.Relu, bias=b2col, scale=1.0
                            )
                        else:
                            nc.vector.tensor_scalar(
                                o[:, :],
                                ps[:, :],
                                b2col,
                                0.0,
                                mybir.AluOpType.add,
                                mybir.AluOpType.max,
                            )
                        nc.sync.dma_start(od, o[:, :])
                    relu_ctr += 1
            ab_cur = ab_next
    nc.compile()
    return nc


def kernel(X, Y, W1, b1, w2, b2):
    global LAST_RESULT, _CACHED_NC
    X = np.ascontiguousarray(np.asarray(X), dtype=np.float32).reshape(B * H, L, D)
    Y = np.ascontiguousarray(np.asarray(Y), dtype=np.float32).reshape(B * H, L, D)
    W1 = np.asarray(W1, dtype=np.float32)
    b1 = np.asarray(b1, dtype=np.float32)
    w2 = np.asarray(w2, dtype=np.float32)
    b2v = float(np.asarray(b2))

    W1T2 = np.ascontiguousarray(
        np.vstack([W1.T, W1.T]).astype(ml_dtypes.bfloat16)
    )
    consts = np.ascontiguousarray(
        np.stack(
            [
                np.tile(b1 * w2, 2),
                np.tile(w2, 2),
                np.tile(b1, 2),
                np.full(128, b2v, np.float32),
            ],
            axis=1,
        ),
        dtype=np.float32,
    )
    ident = np.eye(128, dtype=ml_dtypes.bfloat16)

    if _CACHED_NC is None:
        _CACHED_NC = _build()
    nc = _CACHED_NC

    in_maps = [
        {
            "X": np.ascontiguousarray(X[i * HPC : (i + 1) * HPC]),
            "Y": np.ascontiguousarray(Y[i * HPC : (i + 1) * HPC]),
            "W1T2": W1T2,
            "CONSTS": consts,
            "IDENT": ident,
        }
        for i in range(NCORES)
    ]
    res = run_bass_kernel_spmd(nc, in_maps, list(range(NCORES)))
    LAST_RESULT = res
    out = np.concatenate([res.results[i]["OUT"] for i in range(NCORES)], axis=0)
    return out.reshape(B, H, L, L)
